# revision 39
# baseline (speedup 1.0000x reference)
"""Multi-head attention (B=2, N=2048, C=512, H=8) on 8 trn2 NeuronCores.

Sharding: tensor-parallel over heads x data-parallel over batch.
Core i handles batch b = i//4 and heads {2*(i%4), 2*(i%4)+1} (a contiguous
128-column slice of Wq/Wk/Wv and 128-row slice of Wo). Each core computes
its heads' full attention and a partial output projection; the host sums
the 4 partials per batch (the TP row-split unshard) and stacks batches.

Structure (v3 - bf16 front-end, DMA-heavy phase 1):
  phase 1: SWDGE cast-DMAs (gpsimd queue) load q/kv/pos from DRAM fp32
    straight to SBUF bf16; XBAR DMA-transposes (InstDmaTransposeAnt on the
    SP/ACT HWDGE queues) produce the channel-major layouts - the PE does
    NO transposes and the vector engines do NO copybacks. PE only runs the
    Q/K/V projections (bf16, fp32 PSUM); DVE fuses the pos-add into the
    PSUM->SBUF copy.
  phase 2 (attention, per q-half): software-pipelined S-quad / EXP /
    PV-quad per k-chunk; ACT does nothing but EXP (the binding engine,
    ~1.9us per k-chunk). Denominators ride the PV matmuls as a ones
    column of vp. O' and denominators are copied out of PSUM immediately
    (frees accumulator banks for the next q-half), normalization happens
    in SBUF: reciprocal on DVE, denominator broadcast across partitions
    via a DRAM-bounce DMA on the idle SP queue.
  phase 3 (tail): Y = O128^T Wo as one K=128 matmul per token tile (h1 is
    partition-shifted into O128 rows 64-127 by a DMA during attention),
    bias added on DVE, stores interleaved on SP/ACT queues.

All matmul operands are bf16 (inputs rounded from fp32 by the cast-DMAs /
DVE); accumulation stays fp32 in PSUM. Overall rel err ~2e-3 vs the fp32
reference (gate is 2e-2).
"""
import math

import numpy as np

B, N, C, H, D = 2, 2048, 512, 8, 64
SCALE = float(C) ** -0.5
NCORES = 8
P = 128
# Schraudolph exp via bf16 bit pattern: int16(x*SCHRA_A + SCHRA_B) viewed
# as bf16 approximates exp(x*SCALE) within ~3% (denominators use the same
# approximated values, so the softmax stays exactly normalized).
SCHRA_A = 128.0 / math.log(2.0) * SCALE
SCHRA_B = 128.0 * 127.0 - 722019.0 / 65536.0

_cached_nc = None

# which (h, kc%2) exp tiles go to DVE instead of ACT via the Schraudolph
# bit trick (adds ~3% error on that mass) - empty: all exps exact on ACT
DVE_EXP = set()


def _build():
    from contextlib import ExitStack

    import concourse.mybir as mybir
    import concourse.tile as tile
    from concourse import bacc
    from concourse.tile_rust import add_dep_helper

    f32 = mybir.dt.float32
    bf16 = mybir.dt.bfloat16
    AF = mybir.ActivationFunctionType

    nc = bacc.Bacc("TRN2", target_bir_lowering=False, debug=False)

    qb = nc.dram_tensor("qb", [N, C], f32, kind="ExternalInput")
    kvb = nc.dram_tensor("kvb", [N, C], f32, kind="ExternalInput")
    posq = nc.dram_tensor("posq", [N, D], f32, kind="ExternalInput")
    posk = nc.dram_tensor("posk", [N, D], f32, kind="ExternalInput")
    wq = nc.dram_tensor("wq", [C, P], f32, kind="ExternalInput")
    wk = nc.dram_tensor("wk", [C, P], f32, kind="ExternalInput")
    wv = nc.dram_tensor("wv", [C, P], f32, kind="ExternalInput")
    wo = nc.dram_tensor("wo", [P, C], f32, kind="ExternalInput")
    bo4 = nc.dram_tensor("bo4", [C], f32, kind="ExternalInput")
    eye = nc.dram_tensor("eye", [P, P], f32, kind="ExternalInput")
    y = nc.dram_tensor("y", [N, C], f32, kind="ExternalOutput")

    NT16 = N // P          # 16 token tiles of 128
    NT4 = N // 512         # 4 token chunks of 512
    CC = C // P            # 4 channel chunks of 128

    with tile.TileContext(nc) as tc, ExitStack() as ctx:
        persist = ctx.enter_context(tc.tile_pool(name="persist", bufs=1))
        io = ctx.enter_context(tc.tile_pool(name="io", bufs=4))

        # ---- small input DMAs ----
        # weights (SP queue, fp32), bias broadcast (SWDGE)
        w_raw = {}
        for name, wt in (("wq", wq), ("wk", wk), ("wv", wv)):
            raw = io.tile([P, CC, P], f32, tag="wraw", name=f"wraw_{name}")
            nc.sync.dma_start(raw[:], wt.rearrange("(c p) m -> p c m", p=P))
            w_raw[name] = raw
        wo_raw = io.tile([P, C], f32, tag="wo_raw", name="wo_raw")
        nc.sync.dma_start(wo_raw[:], wo[:])
        bo_bc = persist.tile([P, C], f32, tag="bo_bc")
        nc.gpsimd.dma_start(bo_bc[:], bo4[:].unsqueeze(0).to_broadcast((P, C)))

        # weights to bf16 (DVE rounds)
        w_r = {}
        for name in ("wq", "wk", "wv"):
            wr = persist.tile([P, CC, P], bf16, tag=f"{name}_r")
            nc.vector.tensor_copy(wr[:], w_raw[name][:])
            w_r[name] = wr
        wo_r = persist.tile([P, C], bf16, tag="wo_r")
        nc.vector.tensor_copy(wo_r[:], wo_raw[:])

        # ---- pos: fp32 load, DVE cast, PE transposes (bf16) ----
        # pos_stack[d, n] = pos[n, d], duplicated to partitions 64-127 (the
        # same additive bias applies to both heads of the pair).
        pos_loads = {}
        for name, pt_dram in (("q", posq), ("k", posk)):
            pf = io.tile([P, NT16, D], f32, tag="pos_f", name=f"pos_f{name}")
            nc.scalar.dma_start(
                pf[:], pt_dram.rearrange("(a p) d -> p a d", p=P))
            pos_loads[name] = pf

        # ---- q/kv: SWDGE cast-load slabs + XBAR transposes + projections --
        qhT = persist.tile([P, N], bf16, tag="qhT")
        khT = persist.tile([P, N], bf16, tag="khT")
        vp_sb = persist.tile([P, NT16, 130], bf16, tag="vp")

        # ones columns of vp_sb (denominator trick)
        ones_raw = persist.tile([P, 1], f32, tag="ones")
        nc.gpsimd.memset(ones_raw[:], 1.0)
        ones_bc = ones_raw[:].to_broadcast((P, NT16, 1))
        nc.vector.tensor_copy(vp_sb[:, :, 64:65], ones_bc)
        nc.vector.tensor_copy(vp_sb[:, :, 129:130], ones_bc)

        # bf16 identity for PE transposes
        identf = io.tile([P, P], f32, tag="identf", name="identf")
        nc.sync.dma_start(identf[:], eye[:])
        ident = persist.tile([P, P], bf16, tag="ident")
        nc.vector.tensor_copy(ident[:], identf[:])

        with (
            tc.tile_pool(name="tp_ps", bufs=3, space="PSUM") as tp_ps,
            tc.tile_pool(name="proj_ps", bufs=3, space="PSUM") as proj_ps,
            tc.tile_pool(name="xtbig", bufs=1) as xtbig,
            tc.tile_pool(name="slabs", bufs=3) as slabs,
        ):
            # slab loads dispatch first (kv on SP, q on ACT)
            def load_slab(name, src, nt, tq):
                slab_f = slabs.tile([P, NT4, C], f32, tag=f"{name}slabf",
                                    name=f"{name}slabf{nt}")
                tq.dma_start(
                    slab_f[:],
                    src[nt * 512:(nt + 1) * 512, :].rearrange(
                        "(a p) c -> p a c", p=P))
                return slab_f

            kv_f = [load_slab("kv", kvb, nt, nc.sync) for nt in range(NT4)]
            q_f = [load_slab("q", qb, nt, nc.scalar) for nt in range(NT4)]

            # pos transposes: [128 tok, 64 d] tiles -> [64 d, 128 tok]
            pos_stack = {}
            for name in ("q", "k"):
                pl = persist.tile([P, NT16, D], bf16,
                                  tag=f"pos_ld_{name}")
                nc.vector.tensor_copy(pl[:], pos_loads[name][:])
                stack = persist.tile([P, N], bf16, tag=f"pos_{name}")
                for t2 in range(NT16 // 4):
                    ps = tp_ps.tile([P, 512], bf16, tag="tpx")
                    for j in range(4):
                        t = t2 * 4 + j
                        nc.tensor.transpose(
                            ps[0:64, j * P:(j + 1) * P], pl[:, t, :],
                            ident[:])
                    nc.vector.tensor_copy(
                        stack[0:64, t2 * 512:(t2 + 1) * 512], ps[0:64, :])
                nc.gpsimd.dma_start(stack[64:128, :], stack[0:64, :])
                pos_stack[name] = stack
            xT = {
                "q": [xtbig.tile([P, CC, 512], bf16, tag=f"qT{i}",
                                 name=f"qT{i}") for i in range(NT4)],
                "kv": [xtbig.tile([P, CC, 512], bf16, tag=f"kvT{i}",
                                  name=f"kvT{i}") for i in range(NT4)],
            }
            vpT = [xtbig.tile([P, 512], bf16, tag=f"vpT{i}",
                              name=f"vpT{i}") for i in range(NT4)]

            def cast_slab(name, slab_f, nt):
                """DVE cast fp32->bf16 (2x all-SBUF mode, ~1.1us)."""
                slab = slabs.tile([P, NT4, C], bf16, tag=f"{name}slab",
                                  name=f"{name}slab{nt}")
                nc.vector.tensor_copy(slab[:], slab_f[:])
                return slab

            def tp_chunk(name, slab, nt):
                """PE transposes (bf16, 1 cyc/row) + cheap bf16 copybacks
                split between DVE (2x mode) and ACT."""
                for j in range(NT4):
                    t = nt * 4 + j
                    tp = tp_ps.tile([P, C], bf16, tag="tpx")
                    for cc in range(CC):
                        nc.tensor.transpose(
                            tp[:, cc * P:(cc + 1) * P],
                            slab[:, j, cc * P:(cc + 1) * P], ident[:])
                    src_ap = tp[:].rearrange("p (c x) -> p c x", c=CC)
                    dst_ap = xT[name][nt][:, :, j * P:(j + 1) * P]
                    if j % 2 == 1:
                        nc.scalar.copy(dst_ap, src_ap)
                    else:
                        nc.vector.tensor_copy(dst_ap, src_ap)

            def proj_chunk(wname, srcT, dst, nt, posn=None):
                sl = slice(nt * 512, (nt + 1) * 512)
                pp = proj_ps.tile([P, 512], f32, tag="proj")
                for cc in range(CC):
                    nc.tensor.matmul(
                        pp[:], w_r[wname][:, cc], xT[srcT][nt][:, cc, :],
                        start=(cc == 0), stop=(cc == CC - 1))
                if posn is not None:
                    nc.vector.tensor_add(
                        out=dst[:, sl], in0=pp[:], in1=pos_stack[posn][:, sl])
                elif dst is vpT:
                    nc.vector.tensor_copy(vpT[nt][:], pp[:])
                else:
                    nc.vector.tensor_copy(dst[:, sl], pp[:])

            def vp_tp_chunk(nt):
                for j in range(NT4):
                    t = nt * 4 + j
                    tp = tp_ps.tile([P, C], bf16, tag="tpx")
                    nc.tensor.transpose(
                        tp[:, 0:P], vpT[nt][:, j * P:(j + 1) * P], ident[:])
                    nc.vector.tensor_copy(vp_sb[:, t, 0:64], tp[:, 0:64])
                    nc.vector.tensor_copy(vp_sb[:, t, 65:129],
                                          tp[:, 64:128])

            # cast + transpose + project, chunk by chunk (casts emitted
            # inline so each chunk's DVE work follows its own cast)
            def do_kv(nt):
                slab = cast_slab("kv", kv_f[nt], nt)
                tp_chunk("kv", slab, nt)
                proj_chunk("wk", "kv", khT, nt, posn="k")
                proj_chunk("wv", "kv", vpT, nt)
                vp_tp_chunk(nt)

            def do_q(nt):
                slab = cast_slab("q", q_f[nt], nt)
                tp_chunk("q", slab, nt)
                proj_chunk("wq", "q", qhT, nt, posn="q")

            do_kv(0)
            do_q(0)
            do_q(1)
            for nt in range(1, NT4):
                do_kv(nt)
            do_q(2)
            do_q(3)

        # ---- phase 2: attention, q-halves outer ----
        # Per (q-half, k-chunk): two [128, 1024] PSUM S^T tiles (one per
        # head); EXP per head writes bf16 SBUF; PV-quads accumulate into
        # per-head [65, 1024] accumulators (ones column -> denominators).
        # Emission is software-pipelined (S-quad kc, exp kc, PV-quad kc-1)
        # and PE order pinned so same-kind matmul quads stay contiguous.
        # one O tile per q-half: Y matmuls for half 0 must not wait on
        # half 1's normalize writes (deps are tile-granular)
        O128h = [persist.tile([P, 1024], bf16, tag=f"O128h{i}",
                              name=f"O128h{i}") for i in range(2)]
        with (
            tc.tile_pool(name="st_ps", bufs=1, space="PSUM") as st_ps,
            tc.tile_pool(name="ot_ps", bufs=1, space="PSUM") as ot_ps,
            tc.tile_pool(name="expp", bufs=4) as expp,
            tc.tile_pool(name="den", bufs=1) as den_pool,
            tc.tile_pool(name="dend", bufs=2, space="DRAM") as den_dram_pool,
        ):
            for qh2 in range(2):
                q_sl = slice(qh2 * 1024, (qh2 + 1) * 1024)
                OT = [ot_ps.tile([65, 1024], f32, tag=f"ot{h}",
                                 name=f"ot{h}") for h in (0, 1)]

                pe_prev = [None]

                def chain(mm):
                    if pe_prev[0] is not None:
                        add_dep_helper(mm.ins, pe_prev[0].ins, sync=False,
                                       reason="pin PE quad order")
                    pe_prev[0] = mm

                def s_quad(kc):
                    sts = [st_ps.tile([P, 1024], f32, tag=f"st{h}",
                                      name=f"st{h}") for h in (0, 1)]
                    for h in (0, 1):
                        for qq in range(2):
                            chain(nc.tensor.matmul(
                                sts[h][:, qq * 512:(qq + 1) * 512],
                                khT[64 * h:64 * h + 64, kc * P:(kc + 1) * P],
                                qhT[64 * h:64 * h + 64,
                                    (qh2 * 2 + qq) * 512:
                                    (qh2 * 2 + qq + 1) * 512],
                                start=True, stop=True))
                    exs = []
                    for h in (0, 1):
                        if (h, kc % 2) in DVE_EXP:
                            exi = expp.tile([P, 1024], mybir.dt.int16,
                                            tag=f"exi{h}", name=f"exi{h}")
                            nc.vector.tensor_scalar(
                                out=exi[:], in0=sts[h][:],
                                scalar1=SCHRA_A, scalar2=SCHRA_B,
                                op0=mybir.AluOpType.mult,
                                op1=mybir.AluOpType.add)
                            exs.append(exi[:].bitcast(bf16))
                        else:
                            ex = expp.tile([P, 1024], bf16, tag=f"ex{h}",
                                           name=f"ex{h}")
                            nc.scalar.activation(ex[:], sts[h][:], AF.Exp,
                                                 scale=SCALE)
                            exs.append(ex[:])
                    return exs

                def pv_quad(kc, exs):
                    for h in (0, 1):
                        for qq in range(2):
                            chain(nc.tensor.matmul(
                                OT[h][:, qq * 512:(qq + 1) * 512],
                                vp_sb[:, kc, 65 * h:65 * h + 65],
                                exs[h][:, qq * 512:(qq + 1) * 512],
                                start=(kc == 0), stop=(kc == NT16 - 1)))

                prev = None
                for kc in range(NT16):
                    ex = s_quad(kc)
                    if prev is not None:
                        pv_quad(kc - 1, prev)
                    prev = ex
                pv_quad(NT16 - 1, prev)

                # Copy O' + denominator rows out of PSUM first (frees the
                # accumulator banks for the next q-half early), normalize in
                # SBUF. Denominator broadcast across partitions via a DRAM
                # bounce on the idle SP queue.
                o_raw = []
                for h in (0, 1):
                    orw = den_pool.tile([65, 1024], f32, tag=f"oraw{h}",
                                        name=f"oraw{h}")
                    nc.vector.tensor_copy(orw[:], OT[h][:])
                    o_raw.append(orw)
                for h in (0, 1):
                    den_d = den_dram_pool.tile([1, 1024], f32, tag="den_d")
                    nc.sync.dma_start(den_d[:], o_raw[h][64:65, :])
                    den_bc = den_pool.tile([64, 1024], f32, tag="den_bc")
                    nc.sync.dma_start(
                        den_bc[:], den_d[:].to_broadcast((64, 1024)))
                    den_rec = den_pool.tile([64, 1024], f32, tag="den_rec")
                    nc.vector.reciprocal_approx_fast(den_rec[:], den_bc[:])
                    if h == 0:
                        nc.vector.tensor_mul(
                            out=O128h[qh2][0:64, :], in0=o_raw[h][0:64, :],
                            in1=den_rec[:])
                    else:
                        o_n1 = den_pool.tile([64, 1024], bf16, tag="o_n1")
                        nc.vector.tensor_mul(
                            out=o_n1[:], in0=o_raw[h][0:64, :],
                            in1=den_rec[:])
                        # shift h1 to partitions 64-127 (only DMA can cross
                        # partitions)
                        nc.sync.dma_start(O128h[qh2][64:128, :], o_n1[:])

        # ---- phase 3: output projection, token-major ----
        with (
            tc.tile_pool(name="y_ps", bufs=4, space="PSUM") as y_ps,
            tc.tile_pool(name="yout", bufs=6) as yout,
        ):
            for t in range(NT16):
                yp = y_ps.tile([P, C], f32, tag="y")
                nc.tensor.matmul(
                    yp[:], O128h[t // 8][:, (t % 8) * P:(t % 8 + 1) * P],
                    wo_r[:], start=True, stop=True)
                ysb = yout.tile([P, C], f32, tag="ysb")
                nc.vector.tensor_add(out=ysb[:], in0=yp[:], in1=bo_bc[:])
                oeng = nc.sync if t % 2 == 0 else nc.scalar
                oeng.dma_start(y[t * P:(t + 1) * P, :], ysb[:])

    nc.finalize()
    return nc


def _in_maps(q, kv, pos_q, pos_k, Wq, Wk, Wv, Wo, bo):
    maps = []
    for i in range(NCORES):
        b, hp = i // 4, i % 4
        cs = P * hp
        maps.append({
            "qb": np.ascontiguousarray(q[b], dtype=np.float32),
            "kvb": np.ascontiguousarray(kv[b], dtype=np.float32),
            "posq": np.ascontiguousarray(pos_q[b], dtype=np.float32),
            "posk": np.ascontiguousarray(pos_k[b], dtype=np.float32),
            "wq": np.ascontiguousarray(Wq[:, cs:cs + P], dtype=np.float32),
            "wk": np.ascontiguousarray(Wk[:, cs:cs + P], dtype=np.float32),
            "wv": np.ascontiguousarray(Wv[:, cs:cs + P], dtype=np.float32),
            "wo": np.ascontiguousarray(Wo[cs:cs + P, :], dtype=np.float32),
            "bo4": np.ascontiguousarray(bo, dtype=np.float32) / 4.0,
            "eye": np.eye(P, dtype=np.float32),
        })
    return maps


def kernel(q, kv, pos_q, pos_k, Wq, Wk, Wv, Wo, bo):
    from concourse.bass_utils import run_bass_kernel_spmd

    global _cached_nc
    if _cached_nc is None:
        _cached_nc = _build()

    args = [np.asarray(a) for a in (q, kv, pos_q, pos_k, Wq, Wk, Wv, Wo, bo)]
    maps = _in_maps(*args)
    res = run_bass_kernel_spmd(_cached_nc, maps, list(range(NCORES)))
    outs = [res.results[i]["y"] for i in range(NCORES)]
    y0 = outs[0] + outs[1] + outs[2] + outs[3]
    y1 = outs[4] + outs[5] + outs[6] + outs[7]
    return np.stack([y0, y1]).astype(np.float32)


# revision 43
# speedup vs baseline: 1.0010x; 1.0010x over previous
"""Multi-head attention (B=2, N=2048, C=512, H=8) on 8 trn2 NeuronCores.

Sharding: tensor-parallel over heads x data-parallel over batch.
Core i handles batch b = i//4 and heads {2*(i%4), 2*(i%4)+1} (a contiguous
128-column slice of Wq/Wk/Wv and 128-row slice of Wo). Each core computes
its heads' full attention and a partial output projection; the host sums
the 4 partials per batch (the TP row-split unshard) and stacks batches.

Structure (v3 - bf16 front-end, DMA-heavy phase 1):
  phase 1: SWDGE cast-DMAs (gpsimd queue) load q/kv/pos from DRAM fp32
    straight to SBUF bf16; XBAR DMA-transposes (InstDmaTransposeAnt on the
    SP/ACT HWDGE queues) produce the channel-major layouts - the PE does
    NO transposes and the vector engines do NO copybacks. PE only runs the
    Q/K/V projections (bf16, fp32 PSUM); DVE fuses the pos-add into the
    PSUM->SBUF copy.
  phase 2 (attention, per q-half): software-pipelined S-quad / EXP /
    PV-quad per k-chunk; ACT does nothing but EXP (the binding engine,
    ~1.9us per k-chunk). Denominators ride the PV matmuls as a ones
    column of vp. O' and denominators are copied out of PSUM immediately
    (frees accumulator banks for the next q-half), normalization happens
    in SBUF: reciprocal on DVE, denominator broadcast across partitions
    via a DRAM-bounce DMA on the idle SP queue.
  phase 3 (tail): Y = O128^T Wo as one K=128 matmul per token tile (h1 is
    partition-shifted into O128 rows 64-127 by a DMA during attention),
    bias added on DVE, stores interleaved on SP/ACT queues.

All matmul operands are bf16 (inputs rounded from fp32 by the cast-DMAs /
DVE); accumulation stays fp32 in PSUM. Overall rel err ~2e-3 vs the fp32
reference (gate is 2e-2).
"""
import math

import numpy as np

B, N, C, H, D = 2, 2048, 512, 8, 64
SCALE = float(C) ** -0.5
NCORES = 8
P = 128
# Schraudolph exp via bf16 bit pattern: int16(x*SCHRA_A + SCHRA_B) viewed
# as bf16 approximates exp(x*SCALE) within ~3% (denominators use the same
# approximated values, so the softmax stays exactly normalized).
SCHRA_A = 128.0 / math.log(2.0) * SCALE
SCHRA_B = 128.0 * 127.0 - 722019.0 / 65536.0

_cached_nc = None

# which (h, kc%2) exp tiles go to DVE instead of ACT via the Schraudolph
# bit trick (adds ~3% error on that mass) - empty: all exps exact on ACT
DVE_EXP = set()


def _build():
    from contextlib import ExitStack

    import concourse.mybir as mybir
    import concourse.tile as tile
    from concourse import bacc
    from concourse.tile_rust import add_dep_helper

    f32 = mybir.dt.float32
    bf16 = mybir.dt.bfloat16
    AF = mybir.ActivationFunctionType

    nc = bacc.Bacc("TRN2", target_bir_lowering=False, debug=False)

    qb = nc.dram_tensor("qb", [N, C], f32, kind="ExternalInput")
    kvb = nc.dram_tensor("kvb", [N, C], f32, kind="ExternalInput")
    posq = nc.dram_tensor("posq", [N, D], f32, kind="ExternalInput")
    posk = nc.dram_tensor("posk", [N, D], f32, kind="ExternalInput")
    wq = nc.dram_tensor("wq", [C, P], f32, kind="ExternalInput")
    wk = nc.dram_tensor("wk", [C, P], f32, kind="ExternalInput")
    wv = nc.dram_tensor("wv", [C, P], f32, kind="ExternalInput")
    wo = nc.dram_tensor("wo", [P, C], f32, kind="ExternalInput")
    bo4 = nc.dram_tensor("bo4", [C], f32, kind="ExternalInput")
    eye = nc.dram_tensor("eye", [P, P], f32, kind="ExternalInput")
    y = nc.dram_tensor("y", [N, C], f32, kind="ExternalOutput")

    NT16 = N // P          # 16 token tiles of 128
    NT4 = N // 512         # 4 token chunks of 512
    CC = C // P            # 4 channel chunks of 128

    with tile.TileContext(nc) as tc, ExitStack() as ctx:
        persist = ctx.enter_context(tc.tile_pool(name="persist", bufs=1))
        io = ctx.enter_context(tc.tile_pool(name="io", bufs=4))

        # ---- small input DMAs ----
        # weights (SP queue, fp32), bias broadcast (SWDGE)
        w_raw = {}
        for name, wt in (("wq", wq), ("wk", wk), ("wv", wv)):
            raw = io.tile([P, CC, P], f32, tag="wraw", name=f"wraw_{name}")
            nc.sync.dma_start(raw[:], wt.rearrange("(c p) m -> p c m", p=P))
            w_raw[name] = raw
        wo_raw = io.tile([P, C], f32, tag="wo_raw", name="wo_raw")
        nc.sync.dma_start(wo_raw[:], wo[:])
        bo_bc = persist.tile([P, C], f32, tag="bo_bc")
        nc.gpsimd.dma_start(bo_bc[:], bo4[:].unsqueeze(0).to_broadcast((P, C)))

        # weights to bf16 (DVE rounds)
        w_r = {}
        for name in ("wq", "wk", "wv"):
            wr = persist.tile([P, CC, P], bf16, tag=f"{name}_r")
            nc.vector.tensor_copy(wr[:], w_raw[name][:])
            w_r[name] = wr
        wo_r = persist.tile([P, C], bf16, tag="wo_r")
        nc.vector.tensor_copy(wo_r[:], wo_raw[:])

        # ---- pos: fp32 load, DVE cast, PE transposes (bf16) ----
        # pos_stack[d, n] = pos[n, d], duplicated to partitions 64-127 (the
        # same additive bias applies to both heads of the pair).
        pos_loads = {}
        for name, pt_dram in (("q", posq), ("k", posk)):
            pf = io.tile([P, NT16, D], f32, tag="pos_f", name=f"pos_f{name}")
            nc.scalar.dma_start(
                pf[:], pt_dram.rearrange("(a p) d -> p a d", p=P))
            pos_loads[name] = pf

        # ---- q/kv: SWDGE cast-load slabs + XBAR transposes + projections --
        qhT = persist.tile([P, N], bf16, tag="qhT")
        khT = persist.tile([P, N], bf16, tag="khT")
        vp_sb = persist.tile([P, NT16, 130], bf16, tag="vp")

        # ones columns of vp_sb (denominator trick)
        ones_raw = persist.tile([P, 1], f32, tag="ones")
        nc.gpsimd.memset(ones_raw[:], 1.0)
        ones_bc = ones_raw[:].to_broadcast((P, NT16, 1))
        nc.vector.tensor_copy(vp_sb[:, :, 64:65], ones_bc)
        nc.vector.tensor_copy(vp_sb[:, :, 129:130], ones_bc)

        # bf16 identity for PE transposes
        identf = io.tile([P, P], f32, tag="identf", name="identf")
        nc.sync.dma_start(identf[:], eye[:])
        ident = persist.tile([P, P], bf16, tag="ident")
        nc.vector.tensor_copy(ident[:], identf[:])

        with (
            tc.tile_pool(name="tp_ps", bufs=3, space="PSUM") as tp_ps,
            tc.tile_pool(name="proj_ps", bufs=3, space="PSUM") as proj_ps,
            tc.tile_pool(name="xtbig", bufs=1) as xtbig,
            tc.tile_pool(name="slabs", bufs=3) as slabs,
        ):
            # slab loads dispatch first (kv on SP, q on ACT)
            def load_slab(name, src, nt, tq):
                slab_f = slabs.tile([P, NT4, C], f32, tag=f"{name}slabf",
                                    name=f"{name}slabf{nt}")
                tq.dma_start(
                    slab_f[:],
                    src[nt * 512:(nt + 1) * 512, :].rearrange(
                        "(a p) c -> p a c", p=P))
                return slab_f

            kv_f = [load_slab("kv", kvb, nt, nc.sync) for nt in range(NT4)]
            q_f = [load_slab("q", qb, nt, nc.scalar) for nt in range(NT4)]

            # pos transposes: [128 tok, 64 d] tiles -> [64 d, 128 tok]
            pos_stack = {}
            for name in ("q", "k"):
                pl = persist.tile([P, NT16, D], bf16,
                                  tag=f"pos_ld_{name}")
                nc.vector.tensor_copy(pl[:], pos_loads[name][:])
                stack = persist.tile([P, N], bf16, tag=f"pos_{name}")
                for t2 in range(NT16 // 4):
                    ps = tp_ps.tile([P, 512], bf16, tag="tpx")
                    for j in range(4):
                        t = t2 * 4 + j
                        nc.tensor.transpose(
                            ps[0:64, j * P:(j + 1) * P], pl[:, t, :],
                            ident[:])
                    nc.vector.tensor_copy(
                        stack[0:64, t2 * 512:(t2 + 1) * 512], ps[0:64, :])
                nc.gpsimd.dma_start(stack[64:128, :], stack[0:64, :])
                pos_stack[name] = stack
            xT = {
                "q": [xtbig.tile([P, CC, 512], bf16, tag=f"qT{i}",
                                 name=f"qT{i}") for i in range(NT4)],
                "kv": [xtbig.tile([P, CC, 512], bf16, tag=f"kvT{i}",
                                  name=f"kvT{i}") for i in range(NT4)],
            }
            vpT = [xtbig.tile([P, 512], bf16, tag=f"vpT{i}",
                              name=f"vpT{i}") for i in range(NT4)]

            def cast_slab(name, slab_f, nt):
                """DVE cast fp32->bf16 (2x all-SBUF mode, ~1.1us)."""
                slab = slabs.tile([P, NT4, C], bf16, tag=f"{name}slab",
                                  name=f"{name}slab{nt}")
                nc.vector.tensor_copy(slab[:], slab_f[:])
                return slab

            def tp_chunk(name, slab, nt):
                """PE transposes (bf16, 1 cyc/row) + cheap bf16 copybacks
                split between DVE (2x mode) and ACT."""
                for j in range(NT4):
                    t = nt * 4 + j
                    tp = tp_ps.tile([P, C], bf16, tag="tpx")
                    for cc in range(CC):
                        nc.tensor.transpose(
                            tp[:, cc * P:(cc + 1) * P],
                            slab[:, j, cc * P:(cc + 1) * P], ident[:])
                    src_ap = tp[:].rearrange("p (c x) -> p c x", c=CC)
                    dst_ap = xT[name][nt][:, :, j * P:(j + 1) * P]
                    if j % 2 == 1:
                        nc.scalar.copy(dst_ap, src_ap)
                    else:
                        nc.vector.tensor_copy(dst_ap, src_ap)

            def proj_chunk(wname, srcT, dst, nt, posn=None):
                sl = slice(nt * 512, (nt + 1) * 512)
                pp = proj_ps.tile([P, 512], f32, tag="proj")
                for cc in range(CC):
                    nc.tensor.matmul(
                        pp[:], w_r[wname][:, cc], xT[srcT][nt][:, cc, :],
                        start=(cc == 0), stop=(cc == CC - 1))
                if posn is not None:
                    nc.vector.tensor_add(
                        out=dst[:, sl], in0=pp[:], in1=pos_stack[posn][:, sl])
                elif dst is vpT:
                    nc.vector.tensor_copy(vpT[nt][:], pp[:])
                else:
                    nc.vector.tensor_copy(dst[:, sl], pp[:])

            def vp_tp_chunk(nt):
                for j in range(NT4):
                    t = nt * 4 + j
                    tp = tp_ps.tile([P, C], bf16, tag="tpx")
                    nc.tensor.transpose(
                        tp[:, 0:P], vpT[nt][:, j * P:(j + 1) * P], ident[:])
                    nc.vector.tensor_copy(vp_sb[:, t, 0:64], tp[:, 0:64])
                    nc.vector.tensor_copy(vp_sb[:, t, 65:129],
                                          tp[:, 64:128])

            # cast + transpose + project, chunk by chunk (casts emitted
            # inline so each chunk's DVE work follows its own cast)
            def do_kv(nt):
                slab = cast_slab("kv", kv_f[nt], nt)
                tp_chunk("kv", slab, nt)
                proj_chunk("wk", "kv", khT, nt, posn="k")
                proj_chunk("wv", "kv", vpT, nt)
                vp_tp_chunk(nt)

            def do_q(nt):
                slab = cast_slab("q", q_f[nt], nt)
                tp_chunk("q", slab, nt)
                proj_chunk("wq", "q", qhT, nt, posn="q")

            do_kv(0)
            do_q(0)
            do_q(1)
            for nt in range(1, NT4):
                do_kv(nt)
            do_q(2)
            do_q(3)

        # ---- phase 2: attention, q-halves outer ----
        # Per (q-half, k-chunk): two [128, 1024] PSUM S^T tiles (one per
        # head); EXP per head writes bf16 SBUF; PV-quads accumulate into
        # per-head [65, 1024] accumulators (ones column -> denominators).
        # Emission is software-pipelined (S-quad kc, exp kc, PV-quad kc-1)
        # and PE order pinned so same-kind matmul quads stay contiguous.
        O128 = persist.tile([P, N], bf16, tag="O128")
        with (
            tc.tile_pool(name="st_ps", bufs=1, space="PSUM") as st_ps,
            tc.tile_pool(name="ot_ps", bufs=1, space="PSUM") as ot_ps,
            tc.tile_pool(name="expp", bufs=4) as expp,
            tc.tile_pool(name="den", bufs=1) as den_pool,
            tc.tile_pool(name="dend", bufs=2, space="DRAM") as den_dram_pool,
        ):
            for qh2 in range(2):
                q_sl = slice(qh2 * 1024, (qh2 + 1) * 1024)
                OT = [ot_ps.tile([65, 1024], f32, tag=f"ot{h}",
                                 name=f"ot{h}") for h in (0, 1)]

                pe_prev = [None]

                def chain(mm):
                    if pe_prev[0] is not None:
                        add_dep_helper(mm.ins, pe_prev[0].ins, sync=False,
                                       reason="pin PE quad order")
                    pe_prev[0] = mm

                def s_quad(kc):
                    sts = [st_ps.tile([P, 1024], f32, tag=f"st{h}",
                                      name=f"st{h}") for h in (0, 1)]
                    for h in (0, 1):
                        for qq in range(2):
                            chain(nc.tensor.matmul(
                                sts[h][:, qq * 512:(qq + 1) * 512],
                                khT[64 * h:64 * h + 64, kc * P:(kc + 1) * P],
                                qhT[64 * h:64 * h + 64,
                                    (qh2 * 2 + qq) * 512:
                                    (qh2 * 2 + qq + 1) * 512],
                                start=True, stop=True))
                    exs = []
                    for h in (0, 1):
                        if (h, kc % 2) in DVE_EXP:
                            exi = expp.tile([P, 1024], mybir.dt.int16,
                                            tag=f"exi{h}", name=f"exi{h}")
                            nc.vector.tensor_scalar(
                                out=exi[:], in0=sts[h][:],
                                scalar1=SCHRA_A, scalar2=SCHRA_B,
                                op0=mybir.AluOpType.mult,
                                op1=mybir.AluOpType.add)
                            exs.append(exi[:].bitcast(bf16))
                        else:
                            ex = expp.tile([P, 1024], bf16, tag=f"ex{h}",
                                           name=f"ex{h}")
                            nc.scalar.activation(ex[:], sts[h][:], AF.Exp,
                                                 scale=SCALE)
                            exs.append(ex[:])
                    return exs

                def pv_quad(kc, exs):
                    for h in (0, 1):
                        for qq in range(2):
                            chain(nc.tensor.matmul(
                                OT[h][:, qq * 512:(qq + 1) * 512],
                                vp_sb[:, kc, 65 * h:65 * h + 65],
                                exs[h][:, qq * 512:(qq + 1) * 512],
                                start=(kc == 0), stop=(kc == NT16 - 1)))

                prev = None
                for kc in range(NT16):
                    ex = s_quad(kc)
                    if prev is not None:
                        pv_quad(kc - 1, prev)
                    prev = ex
                pv_quad(NT16 - 1, prev)

                # Copy O' + denominator rows out of PSUM first (frees the
                # accumulator banks for the next q-half early), normalize in
                # SBUF. Denominator broadcast across partitions via a DRAM
                # bounce on the idle SP queue.
                o_raw = []
                for h in (0, 1):
                    orw = den_pool.tile([65, 1024], f32, tag=f"oraw{h}",
                                        name=f"oraw{h}")
                    nc.vector.tensor_copy(orw[:], OT[h][:])
                    o_raw.append(orw)
                for h in (0, 1):
                    den_d = den_dram_pool.tile([1, 1024], f32, tag="den_d")
                    nc.sync.dma_start(den_d[:], o_raw[h][64:65, :])
                    den_bc = den_pool.tile([64, 1024], f32, tag="den_bc")
                    nc.sync.dma_start(
                        den_bc[:], den_d[:].to_broadcast((64, 1024)))
                    den_rec = den_pool.tile([64, 1024], f32, tag="den_rec")
                    nc.vector.reciprocal_approx_fast(den_rec[:], den_bc[:])
                    if h == 0:
                        nc.vector.tensor_mul(
                            out=O128[0:64, q_sl], in0=o_raw[h][0:64, :],
                            in1=den_rec[:])
                    else:
                        o_n1 = den_pool.tile([64, 1024], bf16, tag="o_n1")
                        nc.vector.tensor_mul(
                            out=o_n1[:], in0=o_raw[h][0:64, :],
                            in1=den_rec[:])
                        # shift h1 to partitions 64-127 (only DMA can cross
                        # partitions)
                        nc.sync.dma_start(O128[64:128, q_sl], o_n1[:])

        # ---- phase 3: output projection, token-major ----
        with (
            tc.tile_pool(name="y_ps", bufs=3, space="PSUM") as y_ps,
            tc.tile_pool(name="yout", bufs=4) as yout,
        ):
            bo_pair = bo_bc[:].unsqueeze(1).to_broadcast((P, 2, C))
            for t2 in range(NT16 // 2):
                yp = y_ps.tile([P, 2, C], f32, tag="y")
                for j in range(2):
                    t = t2 * 2 + j
                    nc.tensor.matmul(
                        yp[:, j, :], O128[:, t * P:(t + 1) * P], wo_r[:],
                        start=True, stop=True)
                ysb = yout.tile([P, 2, C], f32, tag="ysb")
                nc.vector.tensor_add(out=ysb[:], in0=yp[:], in1=bo_pair)
                oeng = nc.sync if t2 % 2 == 0 else nc.scalar
                oeng.dma_start(
                    y[t2 * 256:(t2 + 1) * 256, :].rearrange(
                        "(a p) c -> p a c", p=P), ysb[:])

    nc.finalize()
    return nc


def _in_maps(q, kv, pos_q, pos_k, Wq, Wk, Wv, Wo, bo):
    maps = []
    for i in range(NCORES):
        b, hp = i // 4, i % 4
        cs = P * hp
        maps.append({
            "qb": np.ascontiguousarray(q[b], dtype=np.float32),
            "kvb": np.ascontiguousarray(kv[b], dtype=np.float32),
            "posq": np.ascontiguousarray(pos_q[b], dtype=np.float32),
            "posk": np.ascontiguousarray(pos_k[b], dtype=np.float32),
            "wq": np.ascontiguousarray(Wq[:, cs:cs + P], dtype=np.float32),
            "wk": np.ascontiguousarray(Wk[:, cs:cs + P], dtype=np.float32),
            "wv": np.ascontiguousarray(Wv[:, cs:cs + P], dtype=np.float32),
            "wo": np.ascontiguousarray(Wo[cs:cs + P, :], dtype=np.float32),
            "bo4": np.ascontiguousarray(bo, dtype=np.float32) / 4.0,
            "eye": np.eye(P, dtype=np.float32),
        })
    return maps


def kernel(q, kv, pos_q, pos_k, Wq, Wk, Wv, Wo, bo):
    from concourse.bass_utils import run_bass_kernel_spmd

    global _cached_nc
    if _cached_nc is None:
        _cached_nc = _build()

    args = [np.asarray(a) for a in (q, kv, pos_q, pos_k, Wq, Wk, Wv, Wo, bo)]
    maps = _in_maps(*args)
    res = run_bass_kernel_spmd(_cached_nc, maps, list(range(NCORES)))
    outs = [res.results[i]["y"] for i in range(NCORES)]
    y0 = outs[0] + outs[1] + outs[2] + outs[3]
    y1 = outs[4] + outs[5] + outs[6] + outs[7]
    return np.stack([y0, y1]).astype(np.float32)
